# revision 33
# baseline (speedup 1.0000x reference)
"""Trainium2 Bass kernel for nn_CausalAttention (N=4096, 8 heads, DH=32).

Strategy: head-parallel across 8 NeuronCores (1 head per core).
Per core (v6 = v3 baseline + surgical fixes):
  - bf16 inputs/projections (halved DMA + weight loads; PSUM accumulate f32).
  - Scores computed transposed: S^T[k, q] = K @ Q^T, 512-query blocks,
    3 k-tiles (128 keys) per PSUM group (GS=3, row-group packed K=32
    matmuls at partition offsets 0/32/64 pipeline back-to-back on the PE).
    Diagonal sub-tiles skip their fully-masked column prefix.
  - Max-free softmax: P^T = exp(S / sqrt(32)), strict-causal 0/1 mask
    post-exp (scores O(1): no overflow; -10000 masking underflows to 0).
  - Softmax denominator via ones column appended to V (lhsT [128, 33]).
  - V projection: 4 k-tiles batched into one PSUM bank, single DVE
    bias-add per 512-slice.
  - PV matmuls slice away the causally-dead query prefix on diagonal
    tiles; PV flushing crosses block boundaries (2-group lag) so the PE
    never bursts unpipelined at block ends.
  - Projection chunks (K/Q/V per slice) are spread between attention
    groups so their PSUM-slot serialization hides behind score/PV work.
  - DMA (v6): the small constant tensors are packed into three dram
    tensors (weights bf16 / biases f32 / mask bf16), cutting 7
    descriptor-issue instructions (~4.5us of ring-queue serialization)
    down to 3; input k/q slices keep the v3 interleave (first slices +
    consts at the head of both rings, bulk in ring-FIFO order).  Note:
    DMA fabric is ~350 GB/s aggregate across rings — adding rings does
    NOT add bandwidth, only transfer ORDER matters.
  - exp table warmed via a memset tile (input-independent) so the
    ACT_TABLE_LOAD is done before the first scores.
  - Deep-diagonal groups (>=512 dead columns past subtile 0) split the
    exp per-subtile to skip the causally-dead prefixes.
  - o_ps tiles span 33 partitions, so adjacent blocks double-buffer inside
    ONE psum bank at partition offsets 0/64 (no block-boundary stall).
  - Normalization all on-chip, reading o_ps straight from PSUM: denom row
    -> (native add to SBUF) -> reciprocal_approx_fast [1,512] -> gpsimd
    partition_broadcast -> DVE multiply (PSUM read) -> bf16 out DMA,
    staged across following groups; the final block's two 256-col
    normalize chains run after its last PV (a PSUM acc-group read
    hazard would otherwise stall the PV behind the 2.1us chain).
    Output O^T [32, 4096] bf16; host reshapes.
"""

import math

import numpy as np
import ml_dtypes

import concourse.bass as bass
import concourse.mybir as mybir
from concourse import bacc
from concourse.tile import TileContext
from concourse.bass_utils import run_bass_kernel_spmd

# Problem constants (hardcoded per harness contract).
B, CQ, CK, CH, NH, H, W = 1, 256, 256, 256, 8, 64, 64
DH = CH // NH            # 32
N = H * W                # 4096
QB = 512                 # queries per block
NQB = N // QB            # 8
KT = 128                 # keys per k-tile
NKT = N // KT            # 32
GS = 3                   # k-tiles per S-group (3 PSUM banks per group)
NG = (NKT + GS - 1) // GS  # 11 column-groups in packed kT layout
SCALE = 1.0 / math.sqrt(DH)

F32 = mybir.dt.float32
BF16 = mybir.dt.bfloat16

# packed bf16 const layout (columns)
CB_WK = 0      # [0:256)   wk, 2 ch chunks of 128
CB_WQ = 256    # [256:512) wq
CB_WV = 512    # [512:576) wv, 2 ch chunks of 32
CB_COLS = 576
# packed f32 const layout: [128, 4, 33]; [:, t, 0:32] = bv tile t,
# [:, 0, 32] = bk, [:, 1, 32] = bq

_CACHED_NC = None


def _build():
    nc = bacc.Bacc("TRN2", target_bir_lowering=False, debug=False, num_devices=1)

    qin_d = nc.dram_tensor("qin", [CQ, N], BF16, kind="ExternalInput")
    kin_d = nc.dram_tensor("kin", [CK, N], BF16, kind="ExternalInput")
    cb_d = nc.dram_tensor("cb", [128, CB_COLS], BF16, kind="ExternalInput")
    cf_d = nc.dram_tensor("cf", [128, 4, 33], F32, kind="ExternalInput")
    tm_d = nc.dram_tensor("tm", [128, 512], BF16, kind="ExternalInput")
    out_d = nc.dram_tensor("out", [DH, N], BF16, kind="ExternalOutput")

    with TileContext(nc) as tc:
        with (
            tc.tile_pool(name="constp", bufs=1) as constp,
            tc.tile_pool(name="bigp", bufs=1) as bigp,
            tc.tile_pool(name="workp", bufs=4) as workp,
            tc.tile_pool(name="spool", bufs=2, space="PSUM") as spool,
            tc.tile_pool(name="ppool", bufs=1, space="PSUM") as ppool,
            tc.tile_pool(name="opool", bufs=1, space="PSUM") as opool,
        ):
            kin_sb = bigp.tile([128, 2, N], BF16, name="kin_sb")
            qin_sb = bigp.tile([128, 2, N], BF16, name="qin_sb")
            cb_sb = constp.tile([128, CB_COLS], BF16, name="cb_sb")
            cf_sb = constp.tile([128, 4, 33], F32, name="cf_sb")
            kin_ap = kin_d.ap().rearrange("(c p) n -> p c n", p=128)
            qin_ap = qin_d.ap().rearrange("(c p) n -> p c n", p=128)

            def dma_slice(eng, which, s, ch=None):
                sl = slice(512 * s, 512 * (s + 1))
                c = slice(None) if ch is None else slice(ch, ch + 1)
                if which == "k":
                    eng.dma_start(kin_sb[:, c, sl], kin_ap[:, c, sl])
                else:
                    eng.dma_start(qin_sb[:, c, sl], qin_ap[:, c, sl])

            # ---- priority path split across both rings; bulk follows in
            # ring-FIFO order so it can't steal bus from the priority path ----
            tm_sb = constp.tile([128, 512], BF16, name="tm_sb")
            dma_slice(nc.sync, "k", 0, ch=0)
            dma_slice(nc.gpsimd, "q", 0, ch=0)
            dma_slice(nc.sync, "k", 0, ch=1)
            dma_slice(nc.gpsimd, "q", 0, ch=1)
            nc.sync.dma_start(cb_sb[:], cb_d.ap())
            nc.gpsimd.dma_start(cf_sb[:], cf_d.ap())
            dma_slice(nc.sync, "k", 1)
            dma_slice(nc.gpsimd, "q", 1)
            nc.gpsimd.dma_start(tm_sb[:], tm_d.ap())
            # bulk slices 2-7
            for s in range(2, 8):
                dma_slice(nc.gpsimd if s % 2 else nc.sync, "k", s)
                dma_slice(nc.sync if s % 2 else nc.gpsimd, "q", s)

            # views into the packed const tiles
            def wk_v(ch):
                return cb_sb[:, CB_WK + 128 * ch : CB_WK + 128 * (ch + 1)]

            def wq_v(ch):
                return cb_sb[:, CB_WQ + 128 * ch : CB_WQ + 128 * (ch + 1)]

            def wv_v(ch):
                return cb_sb[:, CB_WV + 32 * ch : CB_WV + 32 * (ch + 1)]

            tm_win = tm_sb[:, 384:512]
            bk_col = cf_sb[:, 0, 32:33]
            bq_col = cf_sb[:, 1, 32:33]
            bv_all = cf_sb[:, :, 0:DH]

            # Warm the exp activation table immediately (input-independent).
            wsrc = workp.tile([1, 1], F32, name="wsrc")
            nc.vector.memset(wsrc[:], 0.0)
            warm = workp.tile([1, 1], F32, name="warm")
            nc.scalar.activation(
                warm[:], wsrc[:], mybir.ActivationFunctionType.Exp
            )

            # kT3[32u+d, 128g+kk] = k^T[d, 128*(3g+u)+kk]  (3-way row packing)
            kT3 = bigp.tile([32 * GS, NG * 128], BF16, name="kT3")
            # qT[32u+d, q] = q^T[d, q] for u=0..3 (4x replicated on partitions)
            qT = bigp.tile([128, N], BF16, name="qT")
            # v_all[kk, t, :DH] = v[128t+kk, :]; col DH is the ones column
            v_all = bigp.tile([128, NKT, 40], BF16, name="v_all")
            nc.vector.memset(v_all[:, :, DH : DH + 1], 1.0)

            # ---- projection chunks (emitted interleaved with attention) ----
            def proj_k(s):
                ksl = slice(512 * s, 512 * (s + 1))
                pj = ppool.tile([128, 512], F32, name="pj", tag="p")
                for ch in range(2):
                    nc.tensor.matmul(
                        pj[:],
                        wk_v(ch),
                        kin_sb[:, ch, ksl],
                        start=(ch == 0),
                        stop=(ch == 1),
                    )
                for ci in range(4):
                    j = 4 * s + ci
                    u, g = j % GS, j // GS
                    nc.vector.tensor_scalar_add(
                        kT3[32 * u : 32 * u + 32, 128 * g : 128 * g + 128],
                        pj[32 * u : 32 * u + 32, 128 * ci : 128 * ci + 128],
                        bk_col[32 * u : 32 * u + 32, :],
                    )

            def proj_q(s):
                ksl = slice(512 * s, 512 * (s + 1))
                # slice 0 uses the (still-free) opool bank so the K and Q
                # projection chains run in parallel at startup
                pool = opool if s == 0 else ppool
                tag = "o" if s == 0 else "p"
                pj = pool.tile([128, 512], F32, name="pj", tag=tag)
                for ch in range(2):
                    nc.tensor.matmul(
                        pj[:],
                        wq_v(ch),
                        qin_sb[:, ch, ksl],
                        start=(ch == 0),
                        stop=(ch == 1),
                    )
                nc.vector.tensor_scalar_add(qT[:, ksl], pj[:], bq_col)

            def proj_v(s):
                pj = ppool.tile([128, 4, DH], F32, name="pj", tag="p")
                for ti in range(4):
                    t = 4 * s + ti
                    nsl = slice(128 * t, 128 * (t + 1))
                    for ch in range(2):
                        nc.tensor.matmul(
                            pj[:, ti, :],
                            kin_sb[:, ch, nsl],
                            wv_v(ch),
                            start=(ch == 0),
                            stop=(ch == 1),
                        )
                nc.vector.tensor_add(
                    v_all[:, 4 * s : 4 * s + 4, 0:DH], pj[:], bv_all
                )

            # ---- tails (all on-chip; staged to hide latency) ----
            def tail_b(st):
                cs_sb = workp.tile([1, 512], F32, name="cs_sb")
                nc.vector.tensor_scalar_add(
                    cs_sb[:], st["o_ps"][DH : DH + 1, :], 1e-30
                )
                csr = workp.tile([1, 512], F32, name="csr")
                nc.vector.reciprocal_approx_fast(csr[:], cs_sb[:])
                st.update(csr=csr)

            def tail_b2(st):
                rep = workp.tile([DH, 512], F32, name="rep")
                nc.gpsimd.partition_broadcast(rep[:], st["csr"][:])
                st.update(rep=rep)

            def tail_c(st):
                qb = st["qb"]
                out_sb = workp.tile([DH, 512], BF16, name="out_sb")
                nc.vector.tensor_mul(out_sb[:], st["o_ps"][0:DH, :], st["rep"][:])
                nc.sync.dma_start(
                    out_d.ap()[:, 512 * qb : 512 * (qb + 1)], out_sb[:]
                )

            # ---- global group stream with cross-block PV pends ----
            stage_q = []     # deferred tail stages, advanced per group
            chunk_q = []     # pending projection chunks
            chunks_added = []  # slices whose proj chunks have been queued
            chunks_popped = [0]
            pends = []       # (qb, g, nsub, p_sb) awaiting PV
            ostate = {}      # qb -> {"o_ps": tile, "first": bool}

            def ngroups(qb):
                return (4 * (qb + 1) + GS - 1) // GS

            def final_half(o_ps, c):
                # normalize+emit one 256-col half of the final block
                csl = slice(256 * c, 256 * (c + 1))
                cs = workp.tile([1, 256], F32, name="csh", bufs=2)
                nc.vector.tensor_scalar_add(cs[:], o_ps[DH : DH + 1, csl], 1e-30)
                csr = workp.tile([1, 256], F32, name="csrh", bufs=2)
                nc.vector.reciprocal_approx_fast(csr[:], cs[:])
                rep = workp.tile([DH, 256], F32, name="repc", bufs=2)
                nc.gpsimd.partition_broadcast(rep[:], csr[:])
                outc = workp.tile([DH, 256], BF16, name="outc", bufs=2)
                nc.vector.tensor_mul(outc[:], o_ps[0:DH, csl], rep[:])
                base = 512 * (NQB - 1) + 256 * c
                nc.sync.dma_start(out_d.ap()[:, base : base + 256], outc[:])

            def flush_one():
                qb, g, nsub, p_sb = pends.pop(0)
                st = ostate.get(qb)
                if st is None:
                    # o_ps tiles only span 33 partitions, so adjacent blocks
                    # double-buffer within ONE psum bank at partition offsets
                    # 0/64 (subtile dep tracking keeps them independent)
                    off = 64 * (qb % 2)
                    st = ostate[qb] = {
                        "o_ps": o_base[off : off + DH + 1, :],
                        "first": True,
                        "qb": qb,
                    }
                o_ps = st["o_ps"]
                last_g = g == ngroups(qb) - 1
                for u in range(nsub):
                    j = GS * g + u
                    o = max(0, 128 * j - 512 * qb)
                    if st["first"]:
                        o = 0  # first matmul must initialize full PSUM
                    nc.tensor.matmul(
                        o_ps[:, o:512],
                        v_all[:, j, 0 : DH + 1],
                        p_sb[:, 512 * u + o : 512 * (u + 1)],
                        start=st["first"],
                        stop=(last_g and u == nsub - 1),
                        skip_group_check=True,
                    )
                    st["first"] = False
                if last_g:
                    ts = st
                    if qb == NQB - 1:
                        # defer normalization until after the last PV so the
                        # PSUM acc-group read hazard can't stall the PV
                        final_o.append(o_ps)
                    else:
                        stage_q.append(lambda ts=ts: tail_b(ts))
                        stage_q.append(lambda ts=ts: tail_b2(ts))
                        stage_q.append(lambda ts=ts: tail_c(ts))
                    del ostate[qb]

            def emit_scores(qb, g):
                nkt_q = 4 * (qb + 1)
                nsub = min(GS, nkt_q - GS * g)
                s_ps = spool.tile([128, GS * 512], F32, name="s_ps", tag="s")
                for u in range(nsub):
                    j = GS * g + u
                    o = max(0, 128 * j - 512 * qb)
                    nc.tensor.matmul(
                        s_ps[:, 512 * u + o : 512 * (u + 1)],
                        kT3[32 * u : 32 * u + 32, 128 * g : 128 * g + 128],
                        qT[32 * u : 32 * u + 32, 512 * qb + o : 512 * (qb + 1)],
                        start=True,
                        stop=True,
                    )
                return (qb, g, nsub, s_ps)

            def emit_exp(qb, g, nsub, s_ps):
                p_sb = workp.tile([128, GS * 512], BF16, name="p_sb", bufs=8)
                offs = [max(0, 128 * (GS * g + u) - 512 * qb)
                        for u in range(nsub)]
                if (qb == NQB - 1 and g == ngroups(qb) - 1) or sum(offs[1:]) >= 512:
                    # the very last group heads the exposed drain chain, and
                    # deep-diagonal groups skip enough dead columns to pay
                    # for the extra ACT instruction overhead: per-subtile exp
                    for u in range(nsub):
                        nc.scalar.activation(
                            p_sb[:, 512 * u + offs[u] : 512 * (u + 1)],
                            s_ps[:, 512 * u + offs[u] : 512 * (u + 1)],
                            mybir.ActivationFunctionType.Exp,
                            scale=SCALE,
                        )
                else:
                    nc.scalar.activation(
                        p_sb[:, offs[0] : 512 * nsub],
                        s_ps[:, offs[0] : 512 * nsub],
                        mybir.ActivationFunctionType.Exp,
                        scale=SCALE,
                    )
                for u in range(nsub):
                    j = GS * g + u
                    o = 128 * j - 512 * qb
                    if o >= 0:  # strict-causal mask on the diagonal window
                        nc.vector.tensor_mul(
                            p_sb[:, 512 * u + o : 512 * u + o + 128],
                            p_sb[:, 512 * u + o : 512 * u + o + 128],
                            tm_win,
                        )
                pends.append((qb, g, nsub, p_sb))

            # prefill: slice-0 projections, then stream blocks with the next
            # slice's chunks spread across group boundaries
            proj_k(0)
            proj_q(0)
            proj_v(0)
            # single psum bank shared by all blocks' o_ps (allocated after
            # proj_q(0)'s opool use so the slot rotation stays acyclic)
            o_base = opool.tile([128, 512], F32, name="o_base", tag="o")
            final_o = []

            def emit_group(qb, g):
                emit_exp(*emit_scores(qb, g))

            for qb in range(NQB):
                # slices 1-3 arrive just in time (chunked per block); by
                # block 2 all inputs have landed, so front-load the rest —
                # projections then finish by ~block 5, and late blocks never
                # wait on kT3/qT/v_all writes
                hi = NQB if qb == 3 else min(qb + 2, NQB)
                while len(chunks_added) < hi:
                    s = len(chunks_added)
                    chunks_added.append(s)
                    if s == 0:
                        continue
                    chunk_q.extend(
                        [
                            lambda s=s: proj_k(s),
                            lambda s=s: proj_q(s),
                            lambda s=s: proj_v(s),
                        ]
                    )
                thr = 1 if qb == NQB - 1 else 2  # drain tighter at the end
                for g in range(ngroups(qb)):
                    emit_group(qb, g)
                    # PV flush first: proj matmuls can stall on the ppool
                    # chain and must not head-block ready PV work in the
                    # in-order PE queue
                    if len(pends) > thr:
                        flush_one()
                    if chunk_q:  # proj adds get DVE priority over tail ops
                        chunks_popped[0] += 1
                        chunk_q.pop(0)()
                    if g == 0 and chunk_q:  # K+Q chunks both in group 0
                        chunks_popped[0] += 1
                        chunk_q.pop(0)()
                    if stage_q:
                        stage_q.pop(0)()
                # small early blocks: make sure the next block's slice is
                # fully projected (don't burst-drain the rest)
                while chunk_q and chunks_popped[0] < 3 * (qb + 1):
                    chunks_popped[0] += 1
                    chunk_q.pop(0)()
            while pends:
                flush_one()
            final_half(final_o[0], 0)
            final_half(final_o[0], 1)
            while stage_q:
                stage_q.pop(0)()

    nc.finalize()
    return nc


def _get_nc():
    global _CACHED_NC
    if _CACHED_NC is None:
        _CACHED_NC = _build()
    return _CACHED_NC


def _prep_in_maps(inputs):
    f = lambda a: np.ascontiguousarray(np.asarray(a, dtype=np.float32))
    bf = lambda a: np.ascontiguousarray(a.astype(ml_dtypes.bfloat16))
    query = bf(f(inputs["query"]).reshape(CQ, N))
    key_feat = bf(f(inputs["key_feat"]).reshape(CK, N))

    def wnorm(v, g):
        v = f(v)
        g = f(g)
        return g[:, None] * v / np.linalg.norm(v, axis=1, keepdims=True)

    wq = wnorm(inputs["vq"], inputs["gq"])
    wk = wnorm(inputs["vk"], inputs["gk"])
    wv = wnorm(inputs["vv"], inputs["gv"])
    bq, bk, bv = f(inputs["bq"]), f(inputs["bk"]), f(inputs["bv"])

    # strict-causal diagonal mask window (same for all heads)
    tm_np = (np.arange(128)[:, None] < (np.arange(512)[None, :] - 384)).astype(
        np.float32
    )

    def pack_w(w_rows):  # [32, C] -> [128, 2*128] partition-major chunks
        wt = np.tile(w_rows.T, (1, 4))  # [C, 128]
        return wt.reshape(2, 128, 128).transpose(1, 0, 2).reshape(128, 256)

    in_maps = []
    for c in range(NH):
        rows = slice(DH * c, DH * (c + 1))
        cb = np.zeros((128, CB_COLS), np.float32)
        cb[:, CB_WK : CB_WK + 256] = pack_w(wk[rows])
        cb[:, CB_WQ : CB_WQ + 256] = pack_w(wq[rows])
        cb[:, CB_WV : CB_WV + 64] = (
            wv[rows].T.reshape(2, 128, 32).transpose(1, 0, 2).reshape(128, 64)
        )
        cf = np.zeros((128, 4, 33), np.float32)
        cf[:, :, 0:32] = bv[rows][None, None, :]
        cf[:, 0, 32] = np.tile(bk[rows], 4)
        cf[:, 1, 32] = np.tile(bq[rows], 4)
        in_maps.append(
            {
                "qin": query,
                "kin": key_feat,
                "cb": bf(cb),
                "cf": np.ascontiguousarray(cf),
                "tm": bf(tm_np),
            }
        )
    return in_maps


def _run(inputs, trace=False, **kwargs):
    nc = _get_nc()
    in_maps = _prep_in_maps(inputs)
    res = None
    for attempt in range(3):
        try:
            res = run_bass_kernel_spmd(
                nc, in_maps, core_ids=list(range(NH)), trace=trace, **kwargs
            )
            break
        except Exception:
            if attempt == 2:
                raise

    out = np.empty((B, CH, H, W), dtype=np.float32)
    for c in range(NH):
        oc = np.asarray(res.results[c]["out"], dtype=np.float32)  # [DH, N]
        out[0, DH * c : DH * (c + 1)] = oc.reshape(DH, H, W)
    return out, res


def kernel(**inputs) -> np.ndarray:
    out, _ = _run(inputs, trace=False)
    return out


# revision 37
# speedup vs baseline: 1.1811x; 1.1811x over previous
"""Trainium2 Bass kernel for nn_CausalAttention (N=4096, 8 heads, DH=32).

Strategy: head-parallel across 8 NeuronCores (1 head per core).
Per core (v6 = v3 baseline + surgical fixes):
  - bf16 inputs/projections (halved DMA + weight loads; PSUM accumulate f32).
  - Scores computed transposed: S^T[k, q] = K @ Q^T, 512-query blocks,
    3 k-tiles (128 keys) per PSUM group (GS=3, row-group packed K=32
    matmuls at partition offsets 0/32/64 pipeline back-to-back on the PE).
    Diagonal sub-tiles skip their fully-masked column prefix.
  - Max-free softmax: P^T = exp(S / sqrt(32)), strict-causal 0/1 mask
    post-exp (scores O(1): no overflow; -10000 masking underflows to 0).
  - Softmax denominator via ones column appended to V (lhsT [128, 33]).
  - V projection: 4 k-tiles batched into one PSUM bank, single DVE
    bias-add per 512-slice.
  - PV matmuls slice away the causally-dead query prefix on diagonal
    tiles; PV flushing crosses block boundaries (2-group lag) so the PE
    never bursts unpipelined at block ends.
  - Projection chunks (K/Q/V per slice) are spread between attention
    groups so their PSUM-slot serialization hides behind score/PV work.
  - DMA (v6): the small constant tensors are packed into three dram
    tensors (weights bf16 / biases f32 / mask bf16), cutting 7
    descriptor-issue instructions (~4.5us of ring-queue serialization)
    down to 3; input k/q slices keep the v3 interleave (first slices +
    consts at the head of both rings, bulk in ring-FIFO order).  Note:
    DMA fabric is ~350 GB/s aggregate across rings — adding rings does
    NOT add bandwidth, only transfer ORDER matters.
  - exp table warmed via a memset tile (input-independent) so the
    ACT_TABLE_LOAD is done before the first scores.
  - Deep-diagonal groups (>=512 dead columns past subtile 0) split the
    exp per-subtile to skip the causally-dead prefixes.
  - o_ps tiles span 33 partitions, so adjacent blocks double-buffer inside
    ONE psum bank at partition offsets 0/64 (no block-boundary stall).
  - Normalization all on-chip, reading o_ps straight from PSUM: denom row
    -> (native add to SBUF) -> reciprocal_approx_fast [1,512] -> gpsimd
    partition_broadcast -> DVE multiply (PSUM read) -> bf16 out DMA,
    staged across following groups; the final block takes an immediate
    256-col-pipelined path.  Output O^T [32, 4096] bf16; host reshapes.
"""

import math

import numpy as np
import ml_dtypes

import concourse.bass as bass
import concourse.mybir as mybir
from concourse import bacc
from concourse.tile import TileContext
from concourse.bass_utils import run_bass_kernel_spmd

# Problem constants (hardcoded per harness contract).
B, CQ, CK, CH, NH, H, W = 1, 256, 256, 256, 8, 64, 64
DH = CH // NH            # 32
N = H * W                # 4096
QB = 512                 # queries per block
NQB = N // QB            # 8
KT = 128                 # keys per k-tile
NKT = N // KT            # 32
GS = 3                   # k-tiles per S-group (3 PSUM banks per group)
NG = (NKT + GS - 1) // GS  # 11 column-groups in packed kT layout
SCALE = 1.0 / math.sqrt(DH)

F32 = mybir.dt.float32
BF16 = mybir.dt.bfloat16

# packed bf16 const layout (columns)
CB_WK = 0      # [0:256)   wk, 2 ch chunks of 128
CB_WQ = 256    # [256:512) wq
CB_WV = 512    # [512:576) wv, 2 ch chunks of 32
CB_COLS = 576
# packed f32 const layout: [128, 4, 33]; [:, t, 0:32] = bv tile t,
# [:, 0, 32] = bk, [:, 1, 32] = bq

_CACHED_NC = None


def _build():
    nc = bacc.Bacc("TRN2", target_bir_lowering=False, debug=False, num_devices=1)

    qin_d = nc.dram_tensor("qin", [CQ, N], BF16, kind="ExternalInput")
    kin_d = nc.dram_tensor("kin", [CK, N], BF16, kind="ExternalInput")
    cb_d = nc.dram_tensor("cb", [128, CB_COLS], BF16, kind="ExternalInput")
    cf_d = nc.dram_tensor("cf", [128, 4, 33], F32, kind="ExternalInput")
    tm_d = nc.dram_tensor("tm", [128, 512], BF16, kind="ExternalInput")
    out_d = nc.dram_tensor("out", [DH, N], BF16, kind="ExternalOutput")

    with TileContext(nc) as tc:
        with (
            tc.tile_pool(name="constp", bufs=1) as constp,
            tc.tile_pool(name="bigp", bufs=1) as bigp,
            tc.tile_pool(name="workp", bufs=4) as workp,
            tc.tile_pool(name="spool", bufs=2, space="PSUM") as spool,
            tc.tile_pool(name="ppool", bufs=1, space="PSUM") as ppool,
            tc.tile_pool(name="opool", bufs=1, space="PSUM") as opool,
        ):
            kin_sb = bigp.tile([128, 2, N], BF16, name="kin_sb")
            qin_sb = bigp.tile([128, 2, N], BF16, name="qin_sb")
            cb_sb = constp.tile([128, CB_COLS], BF16, name="cb_sb")
            cf_sb = constp.tile([128, 4, 33], F32, name="cf_sb")
            kin_ap = kin_d.ap().rearrange("(c p) n -> p c n", p=128)
            qin_ap = qin_d.ap().rearrange("(c p) n -> p c n", p=128)

            def dma_slice(eng, which, s, ch=None):
                sl = slice(512 * s, 512 * (s + 1))
                c = slice(None) if ch is None else slice(ch, ch + 1)
                if which == "k":
                    eng.dma_start(kin_sb[:, c, sl], kin_ap[:, c, sl])
                else:
                    eng.dma_start(qin_sb[:, c, sl], qin_ap[:, c, sl])

            # ---- priority path split across both rings; bulk follows in
            # ring-FIFO order so it can't steal bus from the priority path ----
            tm_sb = constp.tile([128, 512], BF16, name="tm_sb")
            dma_slice(nc.sync, "k", 0, ch=0)
            dma_slice(nc.gpsimd, "q", 0, ch=0)
            dma_slice(nc.sync, "k", 0, ch=1)
            dma_slice(nc.gpsimd, "q", 0, ch=1)
            nc.sync.dma_start(cb_sb[:], cb_d.ap())
            nc.gpsimd.dma_start(cf_sb[:], cf_d.ap())
            dma_slice(nc.sync, "k", 1)
            dma_slice(nc.gpsimd, "q", 1)
            nc.gpsimd.dma_start(tm_sb[:], tm_d.ap())
            # bulk slices 2-7
            for s in range(2, 8):
                dma_slice(nc.gpsimd if s % 2 else nc.sync, "k", s)
                dma_slice(nc.sync if s % 2 else nc.gpsimd, "q", s)

            # views into the packed const tiles
            def wk_v(ch):
                return cb_sb[:, CB_WK + 128 * ch : CB_WK + 128 * (ch + 1)]

            def wq_v(ch):
                return cb_sb[:, CB_WQ + 128 * ch : CB_WQ + 128 * (ch + 1)]

            def wv_v(ch):
                return cb_sb[:, CB_WV + 32 * ch : CB_WV + 32 * (ch + 1)]

            tm_win = tm_sb[:, 384:512]
            bk_col = cf_sb[:, 0, 32:33]
            bq_col = cf_sb[:, 1, 32:33]
            bv_all = cf_sb[:, :, 0:DH]

            # Warm the exp activation table immediately (input-independent).
            wsrc = workp.tile([1, 1], F32, name="wsrc")
            nc.vector.memset(wsrc[:], 0.0)
            warm = workp.tile([1, 1], F32, name="warm")
            nc.scalar.activation(
                warm[:], wsrc[:], mybir.ActivationFunctionType.Exp
            )

            # kT3[32u+d, 128g+kk] = k^T[d, 128*(3g+u)+kk]  (3-way row packing)
            kT3 = bigp.tile([32 * GS, NG * 128], BF16, name="kT3")
            # qT[32u+d, q] = q^T[d, q] for u=0..3 (4x replicated on partitions)
            qT = bigp.tile([128, N], BF16, name="qT")
            # v_all[kk, t, :DH] = v[128t+kk, :]; col DH is the ones column
            v_all = bigp.tile([128, NKT, 40], BF16, name="v_all")
            nc.vector.memset(v_all[:, :, DH : DH + 1], 1.0)

            # ---- projection chunks (emitted interleaved with attention) ----
            def proj_k(s):
                ksl = slice(512 * s, 512 * (s + 1))
                pj = ppool.tile([128, 512], F32, name="pj", tag="p")
                for ch in range(2):
                    nc.tensor.matmul(
                        pj[:],
                        wk_v(ch),
                        kin_sb[:, ch, ksl],
                        start=(ch == 0),
                        stop=(ch == 1),
                    )
                for ci in range(4):
                    j = 4 * s + ci
                    u, g = j % GS, j // GS
                    nc.vector.tensor_scalar_add(
                        kT3[32 * u : 32 * u + 32, 128 * g : 128 * g + 128],
                        pj[32 * u : 32 * u + 32, 128 * ci : 128 * ci + 128],
                        bk_col[32 * u : 32 * u + 32, :],
                    )

            def proj_q(s):
                ksl = slice(512 * s, 512 * (s + 1))
                # slice 0 uses the (still-free) opool bank so the K and Q
                # projection chains run in parallel at startup
                pool = opool if s == 0 else ppool
                tag = "o" if s == 0 else "p"
                pj = pool.tile([128, 512], F32, name="pj", tag=tag)
                for ch in range(2):
                    nc.tensor.matmul(
                        pj[:],
                        wq_v(ch),
                        qin_sb[:, ch, ksl],
                        start=(ch == 0),
                        stop=(ch == 1),
                    )
                nc.vector.tensor_scalar_add(qT[:, ksl], pj[:], bq_col)

            def proj_v(s):
                pj = ppool.tile([128, 4, DH], F32, name="pj", tag="p")
                for ti in range(4):
                    t = 4 * s + ti
                    nsl = slice(128 * t, 128 * (t + 1))
                    for ch in range(2):
                        nc.tensor.matmul(
                            pj[:, ti, :],
                            kin_sb[:, ch, nsl],
                            wv_v(ch),
                            start=(ch == 0),
                            stop=(ch == 1),
                        )
                nc.vector.tensor_add(
                    v_all[:, 4 * s : 4 * s + 4, 0:DH], pj[:], bv_all
                )

            # ---- tails (all on-chip; staged to hide latency) ----
            def tail_b(st):
                cs_sb = workp.tile([1, 512], F32, name="cs_sb")
                nc.vector.tensor_scalar_add(
                    cs_sb[:], st["o_ps"][DH : DH + 1, :], 1e-30
                )
                csr = workp.tile([1, 512], F32, name="csr")
                nc.vector.reciprocal_approx_fast(csr[:], cs_sb[:])
                st.update(csr=csr)

            def tail_b2(st):
                rep = workp.tile([DH, 512], F32, name="rep")
                nc.gpsimd.partition_broadcast(rep[:], st["csr"][:])
                st.update(rep=rep)

            def tail_c(st):
                qb = st["qb"]
                out_sb = workp.tile([DH, 512], BF16, name="out_sb")
                nc.vector.tensor_mul(out_sb[:], st["o_ps"][0:DH, :], st["rep"][:])
                nc.sync.dma_start(
                    out_d.ap()[:, 512 * qb : 512 * (qb + 1)], out_sb[:]
                )

            # ---- global group stream with cross-block PV pends ----
            stage_q = []     # deferred tail stages, advanced per group
            chunk_q = []     # pending projection chunks
            chunks_added = []  # slices whose proj chunks have been queued
            chunks_popped = [0]
            pends = []       # (qb, g, nsub, p_sb) awaiting PV
            ostate = {}      # qb -> {"o_ps": tile, "first": bool}

            def ngroups(qb):
                return (4 * (qb + 1) + GS - 1) // GS

            def final_half(o_ps, c):
                # normalize+emit one 256-col half of the final block
                csl = slice(256 * c, 256 * (c + 1))
                cs = workp.tile([1, 256], F32, name="csh", bufs=2)
                nc.vector.tensor_scalar_add(cs[:], o_ps[DH : DH + 1, csl], 1e-30)
                csr = workp.tile([1, 256], F32, name="csrh", bufs=2)
                nc.vector.reciprocal_approx_fast(csr[:], cs[:])
                rep = workp.tile([DH, 256], F32, name="repc", bufs=2)
                nc.gpsimd.partition_broadcast(rep[:], csr[:])
                outc = workp.tile([DH, 256], BF16, name="outc", bufs=2)
                nc.vector.tensor_mul(outc[:], o_ps[0:DH, csl], rep[:])
                base = 512 * (NQB - 1) + 256 * c
                nc.sync.dma_start(out_d.ap()[:, base : base + 256], outc[:])

            def flush_one():
                qb, g, nsub, p_sb = pends.pop(0)
                st = ostate.get(qb)
                if st is None:
                    # o_ps tiles only span 33 partitions, so adjacent blocks
                    # double-buffer within ONE psum bank at partition offsets
                    # 0/64 (subtile dep tracking keeps them independent)
                    off = 64 * (qb % 2)
                    st = ostate[qb] = {
                        "o_ps": o_base[off : off + DH + 1, :],
                        "first": True,
                        "qb": qb,
                    }
                o_ps = st["o_ps"]
                last_g = g == ngroups(qb) - 1
                for u in range(nsub):
                    j = GS * g + u
                    o = max(0, 128 * j - 512 * qb)
                    if st["first"]:
                        o = 0  # first matmul must initialize full PSUM
                    nc.tensor.matmul(
                        o_ps[:, o:512],
                        v_all[:, j, 0 : DH + 1],
                        p_sb[:, 512 * u + o : 512 * (u + 1)],
                        start=st["first"],
                        stop=(last_g and u == nsub - 1),
                        skip_group_check=True,
                    )
                    st["first"] = False
                if qb == NQB - 1 and g == ngroups(qb) - 2:
                    # cols [0:256] of the final block are already complete —
                    # the last group's tiles (30, 31) only write cols >= 256
                    # — so this half's tail overlaps the final group's work
                    final_half(o_ps, 0)
                if last_g:
                    ts = st
                    if qb == NQB - 1:
                        final_half(o_ps, 1)
                    else:
                        stage_q.append(lambda ts=ts: tail_b(ts))
                        stage_q.append(lambda ts=ts: tail_b2(ts))
                        stage_q.append(lambda ts=ts: tail_c(ts))
                    del ostate[qb]

            def emit_scores(qb, g):
                nkt_q = 4 * (qb + 1)
                nsub = min(GS, nkt_q - GS * g)
                s_ps = spool.tile([128, GS * 512], F32, name="s_ps", tag="s")
                for u in range(nsub):
                    j = GS * g + u
                    o = max(0, 128 * j - 512 * qb)
                    nc.tensor.matmul(
                        s_ps[:, 512 * u + o : 512 * (u + 1)],
                        kT3[32 * u : 32 * u + 32, 128 * g : 128 * g + 128],
                        qT[32 * u : 32 * u + 32, 512 * qb + o : 512 * (qb + 1)],
                        start=True,
                        stop=True,
                    )
                return (qb, g, nsub, s_ps)

            def emit_exp(qb, g, nsub, s_ps):
                p_sb = workp.tile([128, GS * 512], BF16, name="p_sb", bufs=8)
                offs = [max(0, 128 * (GS * g + u) - 512 * qb)
                        for u in range(nsub)]
                if (qb == NQB - 1 and g == ngroups(qb) - 1) or sum(offs[1:]) >= 512:
                    # the very last group heads the exposed drain chain, and
                    # deep-diagonal groups skip enough dead columns to pay
                    # for the extra ACT instruction overhead: per-subtile exp
                    for u in range(nsub):
                        nc.scalar.activation(
                            p_sb[:, 512 * u + offs[u] : 512 * (u + 1)],
                            s_ps[:, 512 * u + offs[u] : 512 * (u + 1)],
                            mybir.ActivationFunctionType.Exp,
                            scale=SCALE,
                        )
                else:
                    nc.scalar.activation(
                        p_sb[:, offs[0] : 512 * nsub],
                        s_ps[:, offs[0] : 512 * nsub],
                        mybir.ActivationFunctionType.Exp,
                        scale=SCALE,
                    )
                for u in range(nsub):
                    j = GS * g + u
                    o = 128 * j - 512 * qb
                    if o >= 0:  # strict-causal mask on the diagonal window
                        nc.vector.tensor_mul(
                            p_sb[:, 512 * u + o : 512 * u + o + 128],
                            p_sb[:, 512 * u + o : 512 * u + o + 128],
                            tm_win,
                        )
                pends.append((qb, g, nsub, p_sb))

            # prefill: slice-0 projections, then stream blocks with the next
            # slice's chunks spread across group boundaries
            proj_k(0)
            proj_q(0)
            proj_v(0)
            # single psum bank shared by all blocks' o_ps (allocated after
            # proj_q(0)'s opool use so the slot rotation stays acyclic)
            o_base = opool.tile([128, 512], F32, name="o_base", tag="o")
            def emit_group(qb, g):
                emit_exp(*emit_scores(qb, g))

            for qb in range(NQB):
                # slices 1-3 arrive just in time (chunked per block); by
                # block 2 all inputs have landed, so front-load the rest —
                # projections then finish by ~block 5, and late blocks never
                # wait on kT3/qT/v_all writes
                hi = NQB if qb == 3 else min(qb + 2, NQB)
                while len(chunks_added) < hi:
                    s = len(chunks_added)
                    chunks_added.append(s)
                    if s == 0:
                        continue
                    chunk_q.extend(
                        [
                            lambda s=s: proj_k(s),
                            lambda s=s: proj_q(s),
                            lambda s=s: proj_v(s),
                        ]
                    )
                thr = 1 if qb == NQB - 1 else 2  # drain tighter at the end
                for g in range(ngroups(qb)):
                    emit_group(qb, g)
                    # PV flush first: proj matmuls can stall on the ppool
                    # chain and must not head-block ready PV work in the
                    # in-order PE queue
                    if len(pends) > thr:
                        flush_one()
                    if chunk_q:  # proj adds get DVE priority over tail ops
                        chunks_popped[0] += 1
                        chunk_q.pop(0)()
                    if g == 0 and chunk_q:  # K+Q chunks both in group 0
                        chunks_popped[0] += 1
                        chunk_q.pop(0)()
                    if stage_q:
                        stage_q.pop(0)()
                # small early blocks: make sure the next block's slice is
                # fully projected (don't burst-drain the rest)
                while chunk_q and chunks_popped[0] < 3 * (qb + 1):
                    chunks_popped[0] += 1
                    chunk_q.pop(0)()
            while pends:
                flush_one()
            while stage_q:
                stage_q.pop(0)()

    nc.finalize()
    return nc


def _get_nc():
    global _CACHED_NC
    if _CACHED_NC is None:
        _CACHED_NC = _build()
    return _CACHED_NC


def _prep_in_maps(inputs):
    f = lambda a: np.ascontiguousarray(np.asarray(a, dtype=np.float32))
    bf = lambda a: np.ascontiguousarray(a.astype(ml_dtypes.bfloat16))
    query = bf(f(inputs["query"]).reshape(CQ, N))
    key_feat = bf(f(inputs["key_feat"]).reshape(CK, N))

    def wnorm(v, g):
        v = f(v)
        g = f(g)
        return g[:, None] * v / np.linalg.norm(v, axis=1, keepdims=True)

    wq = wnorm(inputs["vq"], inputs["gq"])
    wk = wnorm(inputs["vk"], inputs["gk"])
    wv = wnorm(inputs["vv"], inputs["gv"])
    bq, bk, bv = f(inputs["bq"]), f(inputs["bk"]), f(inputs["bv"])

    # strict-causal diagonal mask window (same for all heads)
    tm_np = (np.arange(128)[:, None] < (np.arange(512)[None, :] - 384)).astype(
        np.float32
    )

    def pack_w(w_rows):  # [32, C] -> [128, 2*128] partition-major chunks
        wt = np.tile(w_rows.T, (1, 4))  # [C, 128]
        return wt.reshape(2, 128, 128).transpose(1, 0, 2).reshape(128, 256)

    in_maps = []
    for c in range(NH):
        rows = slice(DH * c, DH * (c + 1))
        cb = np.zeros((128, CB_COLS), np.float32)
        cb[:, CB_WK : CB_WK + 256] = pack_w(wk[rows])
        cb[:, CB_WQ : CB_WQ + 256] = pack_w(wq[rows])
        cb[:, CB_WV : CB_WV + 64] = (
            wv[rows].T.reshape(2, 128, 32).transpose(1, 0, 2).reshape(128, 64)
        )
        cf = np.zeros((128, 4, 33), np.float32)
        cf[:, :, 0:32] = bv[rows][None, None, :]
        cf[:, 0, 32] = np.tile(bk[rows], 4)
        cf[:, 1, 32] = np.tile(bq[rows], 4)
        in_maps.append(
            {
                "qin": query,
                "kin": key_feat,
                "cb": bf(cb),
                "cf": np.ascontiguousarray(cf),
                "tm": bf(tm_np),
            }
        )
    return in_maps


def _run(inputs, trace=False, **kwargs):
    nc = _get_nc()
    in_maps = _prep_in_maps(inputs)
    res = None
    for attempt in range(3):
        try:
            res = run_bass_kernel_spmd(
                nc, in_maps, core_ids=list(range(NH)), trace=trace, **kwargs
            )
            break
        except Exception:
            if attempt == 2:
                raise

    out = np.empty((B, CH, H, W), dtype=np.float32)
    for c in range(NH):
        oc = np.asarray(res.results[c]["out"], dtype=np.float32)  # [DH, N]
        out[0, DH * c : DH * (c + 1)] = oc.reshape(DH, H, W)
    return out, res


def kernel(**inputs) -> np.ndarray:
    out, _ = _run(inputs, trace=False)
    return out


# revision 45
# speedup vs baseline: 1.2086x; 1.0233x over previous
"""Trainium2 Bass kernel for nn_CausalAttention (N=4096, 8 heads, DH=32).

Strategy: head-parallel across 8 NeuronCores (1 head per core).
Per core (v6 = v3 baseline + surgical fixes):
  - bf16 inputs/projections (halved DMA + weight loads; PSUM accumulate f32).
  - Scores computed transposed: S^T[k, q] = K @ Q^T, 512-query blocks,
    3 k-tiles (128 keys) per PSUM group (GS=3, row-group packed K=32
    matmuls at partition offsets 0/32/64 pipeline back-to-back on the PE).
    Diagonal sub-tiles skip their fully-masked column prefix.
  - Max-free softmax: P^T = exp(S / sqrt(32)), strict-causal 0/1 mask
    post-exp (scores O(1): no overflow; -10000 masking underflows to 0).
  - Softmax denominator via ones column appended to V (lhsT [128, 33]).
  - V projection: 4 k-tiles batched into one PSUM bank, single DVE
    bias-add per 512-slice.
  - PV matmuls slice away the causally-dead query prefix on diagonal
    tiles; PV flushing crosses block boundaries (2-group lag) so the PE
    never bursts unpipelined at block ends.
  - Projection chunks (K/Q/V per slice) are spread between attention
    groups so their PSUM-slot serialization hides behind score/PV work.
  - DMA (v6): the small constant tensors are packed into three dram
    tensors (weights bf16 / biases f32 / mask bf16), cutting 7
    descriptor-issue instructions (~4.5us of ring-queue serialization)
    down to 3; input k/q slices keep the v3 interleave (first slices +
    consts at the head of both rings, bulk in ring-FIFO order).  Note:
    DMA fabric is ~350 GB/s aggregate across rings — adding rings does
    NOT add bandwidth, only transfer ORDER matters.
  - exp table warmed via a memset tile (input-independent) so the
    ACT_TABLE_LOAD is done before the first scores.
  - Deep-diagonal groups (>=512 dead columns past subtile 0) split the
    exp per-subtile to skip the causally-dead prefixes.
  - o_ps tiles span 33 partitions, so adjacent blocks double-buffer inside
    ONE psum bank at partition offsets 0/64 (no block-boundary stall).
  - Per block the f32 numerator + denominator row (33 partitions) is
    copied PSUM->SBUF in one DVE op and DMA'd out; the softmax division
    happens during the host-side gather in exact f32.  This replaces the
    4-hop on-chip chain (add -> reciprocal_approx -> gpsimd broadcast ->
    multiply, ~2.6us/block of DVE+GpSimd queue time) whose congestion
    starved the exp stream mid-run and serialized the drain.
    Output [33, 4096] f32 per core; host divides + reshapes.
"""

import math

import numpy as np
import ml_dtypes

import concourse.bass as bass
import concourse.mybir as mybir
from concourse import bacc
from concourse.tile import TileContext
from concourse.bass_utils import run_bass_kernel_spmd

# Problem constants (hardcoded per harness contract).
B, CQ, CK, CH, NH, H, W = 1, 256, 256, 256, 8, 64, 64
DH = CH // NH            # 32
N = H * W                # 4096
QB = 512                 # queries per block
NQB = N // QB            # 8
KT = 128                 # keys per k-tile
NKT = N // KT            # 32
GS = 3                   # k-tiles per S-group (3 PSUM banks per group)
NG = (NKT + GS - 1) // GS  # 11 column-groups in packed kT layout
SCALE = 1.0 / math.sqrt(DH)

F32 = mybir.dt.float32
BF16 = mybir.dt.bfloat16

# packed bf16 const layout (columns)
CB_WK = 0      # [0:256)   wk, 2 ch chunks of 128
CB_WQ = 256    # [256:512) wq
CB_WV = 512    # [512:576) wv, 2 ch chunks of 32
CB_COLS = 576
# packed f32 const layout: [128, 4, 33]; [:, t, 0:32] = bv tile t,
# [:, 0, 32] = bk, [:, 1, 32] = bq

_CACHED_NC = None


def _build():
    nc = bacc.Bacc("TRN2", target_bir_lowering=False, debug=False, num_devices=1)

    qin_d = nc.dram_tensor("qin", [CQ, N], BF16, kind="ExternalInput")
    kin_d = nc.dram_tensor("kin", [CK, N], BF16, kind="ExternalInput")
    cb_d = nc.dram_tensor("cb", [128, CB_COLS], BF16, kind="ExternalInput")
    cf_d = nc.dram_tensor("cf", [128, 4, 33], F32, kind="ExternalInput")
    tm_d = nc.dram_tensor("tm", [128, 512], BF16, kind="ExternalInput")
    out_d = nc.dram_tensor("out", [DH + 1, N], F32, kind="ExternalOutput")

    with TileContext(nc) as tc:
        with (
            tc.tile_pool(name="constp", bufs=1) as constp,
            tc.tile_pool(name="bigp", bufs=1) as bigp,
            tc.tile_pool(name="workp", bufs=4) as workp,
            tc.tile_pool(name="spool", bufs=2, space="PSUM") as spool,
            tc.tile_pool(name="ppool", bufs=1, space="PSUM") as ppool,
            tc.tile_pool(name="opool", bufs=1, space="PSUM") as opool,
        ):
            kin_sb = bigp.tile([128, 2, N], BF16, name="kin_sb")
            qin_sb = bigp.tile([128, 2, N], BF16, name="qin_sb")
            cb_sb = constp.tile([128, CB_COLS], BF16, name="cb_sb")
            cf_sb = constp.tile([128, 4, 33], F32, name="cf_sb")
            kin_ap = kin_d.ap().rearrange("(c p) n -> p c n", p=128)
            qin_ap = qin_d.ap().rearrange("(c p) n -> p c n", p=128)

            def dma_slice(eng, which, s, ch=None):
                sl = slice(512 * s, 512 * (s + 1))
                c = slice(None) if ch is None else slice(ch, ch + 1)
                if which == "k":
                    eng.dma_start(kin_sb[:, c, sl], kin_ap[:, c, sl])
                else:
                    eng.dma_start(qin_sb[:, c, sl], qin_ap[:, c, sl])

            # ---- priority path split across both rings; bulk follows in
            # ring-FIFO order so it can't steal bus from the priority path ----
            tm_sb = constp.tile([128, 512], BF16, name="tm_sb")
            dma_slice(nc.sync, "k", 0, ch=0)
            dma_slice(nc.gpsimd, "q", 0, ch=0)
            dma_slice(nc.sync, "k", 0, ch=1)
            dma_slice(nc.gpsimd, "q", 0, ch=1)
            nc.sync.dma_start(cb_sb[:], cb_d.ap())
            nc.gpsimd.dma_start(cf_sb[:], cf_d.ap())
            dma_slice(nc.sync, "k", 1)
            dma_slice(nc.gpsimd, "q", 1)
            nc.gpsimd.dma_start(tm_sb[:], tm_d.ap())
            # bulk slices 2-7
            for s in range(2, 8):
                dma_slice(nc.gpsimd if s % 2 else nc.sync, "k", s)
                dma_slice(nc.sync if s % 2 else nc.gpsimd, "q", s)

            # views into the packed const tiles
            def wk_v(ch):
                return cb_sb[:, CB_WK + 128 * ch : CB_WK + 128 * (ch + 1)]

            def wq_v(ch):
                return cb_sb[:, CB_WQ + 128 * ch : CB_WQ + 128 * (ch + 1)]

            def wv_v(ch):
                return cb_sb[:, CB_WV + 32 * ch : CB_WV + 32 * (ch + 1)]

            tm_win = tm_sb[:, 384:512]
            bk_col = cf_sb[:, 0, 32:33]
            bq_col = cf_sb[:, 1, 32:33]
            bv_all = cf_sb[:, :, 0:DH]

            # Warm the exp activation table immediately (input-independent).
            wsrc = workp.tile([1, 1], F32, name="wsrc")
            nc.vector.memset(wsrc[:], 0.0)
            warm = workp.tile([1, 1], F32, name="warm")
            nc.scalar.activation(
                warm[:], wsrc[:], mybir.ActivationFunctionType.Exp
            )

            # kT3[32u+d, 128g+kk] = k^T[d, 128*(3g+u)+kk]  (3-way row packing)
            kT3 = bigp.tile([32 * GS, NG * 128], BF16, name="kT3")
            # qT[32u+d, q] = q^T[d, q] for u=0..3 (4x replicated on partitions)
            qT = bigp.tile([128, N], BF16, name="qT")
            # v_all[kk, t, :DH] = v[128t+kk, :]; col DH is the ones column
            v_all = bigp.tile([128, NKT, 40], BF16, name="v_all")
            nc.vector.memset(v_all[:, :, DH : DH + 1], 1.0)

            # ---- projection chunks (emitted interleaved with attention) ----
            def proj_k(s):
                ksl = slice(512 * s, 512 * (s + 1))
                pj = ppool.tile([128, 512], F32, name="pj", tag="p")
                for ch in range(2):
                    nc.tensor.matmul(
                        pj[:],
                        wk_v(ch),
                        kin_sb[:, ch, ksl],
                        start=(ch == 0),
                        stop=(ch == 1),
                    )
                for ci in range(4):
                    j = 4 * s + ci
                    u, g = j % GS, j // GS
                    nc.vector.tensor_scalar_add(
                        kT3[32 * u : 32 * u + 32, 128 * g : 128 * g + 128],
                        pj[32 * u : 32 * u + 32, 128 * ci : 128 * ci + 128],
                        bk_col[32 * u : 32 * u + 32, :],
                    )

            def proj_q(s):
                ksl = slice(512 * s, 512 * (s + 1))
                # slice 0 uses the (still-free) opool bank so the K and Q
                # projection chains run in parallel at startup
                pool = opool if s == 0 else ppool
                tag = "o" if s == 0 else "p"
                pj = pool.tile([128, 512], F32, name="pj", tag=tag)
                for ch in range(2):
                    nc.tensor.matmul(
                        pj[:],
                        wq_v(ch),
                        qin_sb[:, ch, ksl],
                        start=(ch == 0),
                        stop=(ch == 1),
                    )
                nc.vector.tensor_scalar_add(qT[:, ksl], pj[:], bq_col)

            def proj_v(s):
                pj = ppool.tile([128, 4, DH], F32, name="pj", tag="p")
                for ti in range(4):
                    t = 4 * s + ti
                    nsl = slice(128 * t, 128 * (t + 1))
                    for ch in range(2):
                        nc.tensor.matmul(
                            pj[:, ti, :],
                            kin_sb[:, ch, nsl],
                            wv_v(ch),
                            start=(ch == 0),
                            stop=(ch == 1),
                        )
                nc.vector.tensor_add(
                    v_all[:, 4 * s : 4 * s + 4, 0:DH], pj[:], bv_all
                )

            # ---- block tail: ship the f32 numerator + denominator row
            # straight out; the softmax division happens during the host
            # gather (exact f32, replacing the 4-hop on-chip chain of
            # add -> reciprocal_approx -> gpsimd broadcast -> multiply
            # whose DVE/GpSimd queue time starved the exp stream) ----
            def tail_out(o_ps, qb):
                out_sb = workp.tile([DH + 1, 512], F32, name="out_sb", bufs=2)
                nc.vector.tensor_copy(out_sb[:], o_ps[:, :])
                nc.sync.dma_start(
                    out_d.ap()[:, 512 * qb : 512 * (qb + 1)], out_sb[:]
                )

            # ---- global group stream with cross-block PV pends ----
            chunk_q = []     # pending projection chunks
            chunks_added = []  # slices whose proj chunks have been queued
            chunks_popped = [0]
            pends = []       # (qb, g, nsub, p_sb) awaiting PV
            ostate = {}      # qb -> {"o_ps": tile, "first": bool}

            def ngroups(qb):
                return (4 * (qb + 1) + GS - 1) // GS

            def flush_one():
                qb, g, nsub, p_sb = pends.pop(0)
                st = ostate.get(qb)
                if st is None:
                    # o_ps tiles only span 33 partitions, so adjacent blocks
                    # double-buffer within ONE psum bank at partition offsets
                    # 0/64 (subtile dep tracking keeps them independent)
                    off = 64 * (qb % 2)
                    st = ostate[qb] = {
                        "o_ps": o_base[off : off + DH + 1, :],
                        "first": True,
                        "qb": qb,
                    }
                o_ps = st["o_ps"]
                last_g = g == ngroups(qb) - 1
                for u in range(nsub):
                    j = GS * g + u
                    o = max(0, 128 * j - 512 * qb)
                    if st["first"]:
                        o = 0  # first matmul must initialize full PSUM
                    nc.tensor.matmul(
                        o_ps[:, o:512],
                        v_all[:, j, 0 : DH + 1],
                        p_sb[:, 512 * u + o : 512 * (u + 1)],
                        start=st["first"],
                        stop=(last_g and u == nsub - 1),
                        skip_group_check=True,
                    )
                    st["first"] = False
                if last_g:
                    tail_out(o_ps, qb)
                    del ostate[qb]

            def emit_scores(qb, g):
                nkt_q = 4 * (qb + 1)
                nsub = min(GS, nkt_q - GS * g)
                s_ps = spool.tile([128, GS * 512], F32, name="s_ps", tag="s")
                for u in range(nsub):
                    j = GS * g + u
                    o = max(0, 128 * j - 512 * qb)
                    nc.tensor.matmul(
                        s_ps[:, 512 * u + o : 512 * (u + 1)],
                        kT3[32 * u : 32 * u + 32, 128 * g : 128 * g + 128],
                        qT[32 * u : 32 * u + 32, 512 * qb + o : 512 * (qb + 1)],
                        start=True,
                        stop=True,
                    )
                return (qb, g, nsub, s_ps)

            def emit_exp(qb, g, nsub, s_ps):
                p_sb = workp.tile([128, GS * 512], BF16, name="p_sb", bufs=8)
                offs = [max(0, 128 * (GS * g + u) - 512 * qb)
                        for u in range(nsub)]
                if (qb == NQB - 1 and g == ngroups(qb) - 1) or sum(offs[1:]) >= 512:
                    # the very last group heads the exposed drain chain, and
                    # deep-diagonal groups skip enough dead columns to pay
                    # for the extra ACT instruction overhead: per-subtile exp
                    for u in range(nsub):
                        nc.scalar.activation(
                            p_sb[:, 512 * u + offs[u] : 512 * (u + 1)],
                            s_ps[:, 512 * u + offs[u] : 512 * (u + 1)],
                            mybir.ActivationFunctionType.Exp,
                            scale=SCALE,
                        )
                else:
                    nc.scalar.activation(
                        p_sb[:, offs[0] : 512 * nsub],
                        s_ps[:, offs[0] : 512 * nsub],
                        mybir.ActivationFunctionType.Exp,
                        scale=SCALE,
                    )
                for u in range(nsub):
                    j = GS * g + u
                    o = 128 * j - 512 * qb
                    if o >= 0:  # strict-causal mask on the diagonal window
                        nc.vector.tensor_mul(
                            p_sb[:, 512 * u + o : 512 * u + o + 128],
                            p_sb[:, 512 * u + o : 512 * u + o + 128],
                            tm_win,
                        )
                pends.append((qb, g, nsub, p_sb))

            # prefill: slice-0 projections, then stream blocks with the next
            # slice's chunks spread across group boundaries
            proj_k(0)
            proj_q(0)
            proj_v(0)
            # single psum bank shared by all blocks' o_ps (allocated after
            # proj_q(0)'s opool use so the slot rotation stays acyclic)
            o_base = opool.tile([128, 512], F32, name="o_base", tag="o")
            def emit_group(qb, g):
                emit_exp(*emit_scores(qb, g))

            for qb in range(NQB):
                # slices 1-3 arrive just in time (chunked per block); by
                # block 2 all inputs have landed, so front-load the rest —
                # projections then finish by ~block 5, and late blocks never
                # wait on kT3/qT/v_all writes
                hi = NQB if qb == 3 else min(qb + 2, NQB)
                while len(chunks_added) < hi:
                    s = len(chunks_added)
                    chunks_added.append(s)
                    if s == 0:
                        continue
                    chunk_q.extend(
                        [
                            lambda s=s: proj_k(s),
                            lambda s=s: proj_q(s),
                            lambda s=s: proj_v(s),
                        ]
                    )
                thr = 1 if qb == NQB - 1 else 2  # drain tighter at the end
                for g in range(ngroups(qb)):
                    emit_group(qb, g)
                    # PV flush first: proj matmuls can stall on the ppool
                    # chain and must not head-block ready PV work in the
                    # in-order PE queue
                    if len(pends) > thr:
                        flush_one()
                    if chunk_q:  # proj adds get DVE priority over tail ops
                        chunks_popped[0] += 1
                        chunk_q.pop(0)()
                    if g == 0 and chunk_q:  # K+Q chunks both in group 0
                        chunks_popped[0] += 1
                        chunk_q.pop(0)()
                # small early blocks: make sure the next block's slice is
                # fully projected (don't burst-drain the rest)
                while chunk_q and chunks_popped[0] < 3 * (qb + 1):
                    chunks_popped[0] += 1
                    chunk_q.pop(0)()
            while pends:
                flush_one()

    nc.finalize()
    return nc


def _get_nc():
    global _CACHED_NC
    if _CACHED_NC is None:
        _CACHED_NC = _build()
    return _CACHED_NC


def _prep_in_maps(inputs):
    f = lambda a: np.ascontiguousarray(np.asarray(a, dtype=np.float32))
    bf = lambda a: np.ascontiguousarray(a.astype(ml_dtypes.bfloat16))
    query = bf(f(inputs["query"]).reshape(CQ, N))
    key_feat = bf(f(inputs["key_feat"]).reshape(CK, N))

    def wnorm(v, g):
        v = f(v)
        g = f(g)
        return g[:, None] * v / np.linalg.norm(v, axis=1, keepdims=True)

    wq = wnorm(inputs["vq"], inputs["gq"])
    wk = wnorm(inputs["vk"], inputs["gk"])
    wv = wnorm(inputs["vv"], inputs["gv"])
    bq, bk, bv = f(inputs["bq"]), f(inputs["bk"]), f(inputs["bv"])

    # strict-causal diagonal mask window (same for all heads)
    tm_np = (np.arange(128)[:, None] < (np.arange(512)[None, :] - 384)).astype(
        np.float32
    )

    def pack_w(w_rows):  # [32, C] -> [128, 2*128] partition-major chunks
        wt = np.tile(w_rows.T, (1, 4))  # [C, 128]
        return wt.reshape(2, 128, 128).transpose(1, 0, 2).reshape(128, 256)

    in_maps = []
    for c in range(NH):
        rows = slice(DH * c, DH * (c + 1))
        cb = np.zeros((128, CB_COLS), np.float32)
        cb[:, CB_WK : CB_WK + 256] = pack_w(wk[rows])
        cb[:, CB_WQ : CB_WQ + 256] = pack_w(wq[rows])
        cb[:, CB_WV : CB_WV + 64] = (
            wv[rows].T.reshape(2, 128, 32).transpose(1, 0, 2).reshape(128, 64)
        )
        cf = np.zeros((128, 4, 33), np.float32)
        cf[:, :, 0:32] = bv[rows][None, None, :]
        cf[:, 0, 32] = np.tile(bk[rows], 4)
        cf[:, 1, 32] = np.tile(bq[rows], 4)
        in_maps.append(
            {
                "qin": query,
                "kin": key_feat,
                "cb": bf(cb),
                "cf": np.ascontiguousarray(cf),
                "tm": bf(tm_np),
            }
        )
    return in_maps


def _run(inputs, trace=False, **kwargs):
    nc = _get_nc()
    in_maps = _prep_in_maps(inputs)
    res = None
    for attempt in range(3):
        try:
            res = run_bass_kernel_spmd(
                nc, in_maps, core_ids=list(range(NH)), trace=trace, **kwargs
            )
            break
        except Exception:
            if attempt == 2:
                raise

    out = np.empty((B, CH, H, W), dtype=np.float32)
    for c in range(NH):
        oc = np.asarray(res.results[c]["out"], dtype=np.float32)  # [DH+1, N]
        # softmax denominator rides along as row DH; exact f32 divide here
        # (query 0 is fully masked: num = den = 0 -> guard keeps it 0)
        norm = oc[0:DH] / np.maximum(oc[DH : DH + 1], 1e-30)
        out[0, DH * c : DH * (c + 1)] = norm.reshape(DH, H, W)
    return out, res


def kernel(**inputs) -> np.ndarray:
    out, _ = _run(inputs, trace=False)
    return out
